# revision 8
# baseline (speedup 1.0000x reference)
"""GPT-OSS MoE layer (E=32 experts, top-4, H=I=1024, T=1024 tokens) on 8 TRN2
NeuronCores.

Expert-parallel sharding (4 experts/core). The host computes the router
dispatch (token->expert assignment) and performs the all-to-all gather/
scatter as part of sharding; every MLP FLOP (gate/up proj, SwiGLU, down
proj, bias adds, combine-weight scaling) runs on device.

This problem is memory-regime: the binding resource is HBM streaming of the
expert weights (48 MB/core in fp32). Weights and activations are therefore
downcast to bf16 on the host (relative error ~0.3% against a 2e-2 budget),
halving HBM traffic to ~27 MB/core. bf16 matmuls run 1 PE cycle/row at any
moving-dim size (no >=256 padding requirement like fp32r), so the moving
dim is the actual routed token capacity C. Weights stream from HBM as
[128, 2048]/[128, 1024] bf16 chunks (4 KB / 2 KB per partition line)
alternating across the two HWDGE queues (sync + scalar engines); token
activations ride SWDGE (gpsimd) as one [128, 8*C] transfer per expert in
and out, keeping descriptor lines >= 2.5 KB.
"""

import os
import sys
import types

import numpy as np
import ml_dtypes

BF16 = ml_dtypes.bfloat16

NUM_EXPERTS = 32
TOP_K = 4
H = 1024
INTER = 1024
N_CORES = 8
EPC = NUM_EXPERTS // N_CORES  # experts per core
P = 128
KT = H // P  # k tiles per contraction (8)


def _install_ntff_hook():
    """Best-effort: restore the NTFF profile hook missing from this image so
    trace=True (or BASS_TRACE=1) in run_bass_kernel_spmd can measure HW time."""
    try:
        from antenv.axon_hooks import get_axon_ntff_profile_hook  # noqa: F401

        return
    except ImportError:
        pass
    try:
        from trn_agent_boot.trn_boot import _ntff_profile_via_ctypes

        hook = _ntff_profile_via_ctypes("/opt/axon/libaxon_pjrt.so")
        mod = types.ModuleType("antenv.axon_hooks")
        mod.get_axon_ntff_profile_hook = lambda: hook
        mod.set_axon_ntff_profile_hook = lambda h: None
        sys.modules["antenv.axon_hooks"] = mod
    except Exception:
        pass


_install_ntff_hook()

_NC_CACHE = {}
last_exec_time_ns = None


def _build_nc(C):
    """Build + compile the per-core Bass program.

    C = token capacity per expert (actual routed max, rounded up to 16).
    All matmuls use moving dim C (bf16 runs full rate at any size).
    """
    import concourse.mybir as mybir
    import concourse.tile as tile
    from concourse import bacc

    dt = mybir.dt.float32
    bt = mybir.dt.bfloat16
    AF = mybir.ActivationFunctionType

    nc = bacc.Bacc(trn_type="TRN2")
    xg = nc.dram_tensor("xg", [EPC, P, KT * C], bt, kind="ExternalInput")
    # mg-major weight layout: each 512-column block's 8 k-chunks are
    # contiguous in the stream, so compute consumes chunks just-in-time and
    # only ~1us of work remains after the last weight byte lands
    w1p = nc.dram_tensor("w1p", [EPC, 4, H, 512], bt, kind="ExternalInput")
    w2t = nc.dram_tensor("w2t", [EPC, 2, INTER, 512], bt, kind="ExternalInput")
    b1p = nc.dram_tensor("b1p", [EPC, P, 16], dt, kind="ExternalInput")
    b2p = nc.dram_tensor("b2p", [EPC, P, 8], dt, kind="ExternalInput")
    ce = nc.dram_tensor("ce", [EPC, C], dt, kind="ExternalInput")
    yT = nc.dram_tensor("yT", [EPC, P, KT * C], bt, kind="ExternalOutput")

    with tile.TileContext(nc) as tc:
        with (
            tc.tile_pool(name="xp", bufs=EPC) as x_pool,
            tc.tile_pool(name="w1", bufs=16) as w1_pool,
            tc.tile_pool(name="w2", bufs=16) as w2_pool,
            tc.tile_pool(name="hp", bufs=16) as h_pool,
            tc.tile_pool(name="ev", bufs=4) as ev_pool,
            tc.tile_pool(name="sm", bufs=EPC) as small_pool,
            tc.tile_pool(name="yo", bufs=2) as y_pool,
            tc.tile_pool(name="ps", bufs=1, space="PSUM") as psum_pool,
        ):
            # hoist ALL token loads + per-expert smalls to the front of the
            # gpsimd queue: nothing later on SWDGE may gate compute (y stores
            # queued behind smalls would serialize the expert pipeline)
            xts, b1ts, b2ts, ce_bs = [], [], [], []
            for e in range(EPC):
                xt = x_pool.tile([P, KT * C], bt, tag="xt", name=f"xt{e}")
                nc.gpsimd.dma_start(xt[:], xg[e])
                xts.append(xt)
            for e in range(EPC):
                b1t = small_pool.tile([P, 16], dt, tag="b1t", name=f"b1t{e}")
                nc.gpsimd.dma_start(b1t[:], b1p[e])
                b1ts.append(b1t)
                b2t = small_pool.tile([P, 8], dt, tag="b2t", name=f"b2t{e}")
                nc.gpsimd.dma_start(b2t[:], b2p[e])
                b2ts.append(b2t)
            ce_rows = []
            for e in range(EPC):
                ce_row = small_pool.tile([1, C], dt, tag="ce_row", name=f"ce_row{e}")
                nc.gpsimd.dma_start(ce_row[:], ce[e : e + 1, :])
                ce_rows.append(ce_row)
            for e in range(EPC):
                # broadcast ce across partitions on gpsimd (keeps PE/PSUM free)
                ce_b = small_pool.tile([P, C], dt, tag="ce_b", name=f"ce_b{e}")
                nc.gpsimd.partition_broadcast(ce_b[:], ce_rows[e][:])
                ce_bs.append(ce_b)

            for e in range(EPC):
                xt = xts[e]
                b1t, b2t, ce_b = b1ts[e], b2ts[e], ce_bs[e]

                # ---- gate/up projection + SwiGLU (tokens in free dim) ----
                # w1p columns are packed in pair-blocks [g0 u0 g1 u1 ...]
                h = []
                for mg in range(4):
                    gps = [
                        psum_pool.tile([P, C], dt, tag=t, name=t)
                        for t in ("g0", "u0", "g1", "u1")
                    ]
                    for k in range(KT):
                        wc = w1_pool.tile([P, 512], bt, tag="w1c", name="w1c")
                        eng = nc.sync if (k % 2 == 0) else nc.scalar
                        eng.dma_start(wc[:], w1p[e, mg, k * P : (k + 1) * P, :])
                        for j in range(4):
                            nc.tensor.matmul(
                                gps[j][:],
                                wc[:, j * P : (j + 1) * P],
                                xt[:, k * C : (k + 1) * C],
                                start=(k == 0),
                                stop=(k == KT - 1),
                            )
                    for pair in range(2):
                        jg = 4 * mg + 2 * pair  # packed block idx of g half
                        sg = ev_pool.tile([P, C], bt, tag="sg")
                        nc.scalar.activation(
                            sg[:],
                            gps[2 * pair][:],
                            AF.Silu,
                            bias=b1t[:, jg : jg + 1],
                        )
                        us = ev_pool.tile([P, C], bt, tag="us")
                        nc.vector.tensor_scalar_add(
                            us[:], gps[2 * pair + 1][:], b1t[:, jg + 1 : jg + 2]
                        )
                        hm = h_pool.tile([P, C], bt, tag="h")
                        nc.vector.tensor_mul(hm[:], sg[:], us[:])
                        h.append(hm)

                # ---- down projection + bias + combine scale ----
                ystage = y_pool.tile([P, KT * C], bt, tag="ystage")
                half = KT * C // 2
                for m2g in range(2):
                    yps = [
                        psum_pool.tile([P, C], dt, tag=f"y{j}", name=f"y{j}")
                        for j in range(4)
                    ]
                    for k in range(KT):
                        wc = w2_pool.tile([P, 512], bt, tag="w2c", name="w2c")
                        eng = nc.scalar if (k % 2 == 0) else nc.sync
                        eng.dma_start(wc[:], w2t[e, m2g, k * P : (k + 1) * P, :])
                        for j in range(4):
                            nc.tensor.matmul(
                                yps[j][:],
                                wc[:, j * P : (j + 1) * P],
                                h[k][:],
                                start=(k == 0),
                                stop=(k == KT - 1),
                            )
                    for j in range(4):
                        m2 = 4 * m2g + j
                        # yo = (y + b2_col) * ce  in one DVE op
                        nc.vector.scalar_tensor_tensor(
                            ystage[:, m2 * C : (m2 + 1) * C],
                            yps[j][:],
                            b2t[:, m2 : m2 + 1],
                            ce_b[:],
                            mybir.AluOpType.add,
                            mybir.AluOpType.mult,
                        )
                    # store each half as soon as its epilogue is done; the
                    # last expert rides the (by then idle) HWDGE queues
                    sl_ = slice(m2g * half, (m2g + 1) * half)
                    if e < EPC - 1:
                        nc.gpsimd.dma_start(yT[e, :, sl_], ystage[:, sl_])
                    else:
                        eng = nc.sync if m2g == 0 else nc.scalar
                        eng.dma_start(yT[e, :, sl_], ystage[:, sl_])

    nc.compile()
    return nc


def _get_nc(C):
    if C not in _NC_CACHE:
        _NC_CACHE[C] = _build_nc(C)
    return _NC_CACHE[C]


_PACK_CACHE = {}


def _w1_col_order():
    # packed column order for w1.T: pair blocks [g_m | u_m] of 128 channels
    return np.concatenate(
        [
            np.r_[m * P : (m + 1) * P, INTER + m * P : INTER + (m + 1) * P]
            for m in range(INTER // P)
        ]
    )


def _pack_weights(w1, b1, w2, b2):
    """Pre-transpose/pack expert weights for the device layout (bf16). Cached
    across calls on a value fingerprint so repeat invocations skip the copy."""
    key = (
        w1.shape,
        w2.shape,
        w1.reshape(-1)[:: 65537][:64].tobytes(),
        w2.reshape(-1)[:: 65537][:64].tobytes(),
        b1.reshape(-1)[:16].tobytes(),
        b2.reshape(-1)[:16].tobytes(),
    )
    if key in _PACK_CACHE:
        return _PACK_CACHE[key]
    col_order = _w1_col_order()
    # mg-major: [E, 4, H, 512] for w1, [E, 2, I, 512] for w2
    w1p_all = np.ascontiguousarray(
        w1.transpose(0, 2, 1)[:, :, col_order]
        .reshape(NUM_EXPERTS, H, 4, 512)
        .transpose(0, 2, 1, 3)
    ).astype(BF16)
    w2t_all = np.ascontiguousarray(
        w2.transpose(0, 2, 1).reshape(NUM_EXPERTS, INTER, 2, 512).transpose(0, 2, 1, 3)
    ).astype(BF16)
    b1p_all = np.ascontiguousarray(
        b1[:, col_order].reshape(NUM_EXPERTS, 16, P).transpose(0, 2, 1)
    )
    b2p_all = np.ascontiguousarray(b2.reshape(NUM_EXPERTS, 8, P).transpose(0, 2, 1))
    _PACK_CACHE[key] = (w1p_all, w2t_all, b1p_all, b2p_all)
    return _PACK_CACHE[key]


def _route(x, wg, bg):
    """Host-side router dispatch: which experts get which tokens, and the
    renormalized combine weights (matches softmax -> top-k -> renorm)."""
    logits = (x.astype(np.float64) @ wg.astype(np.float64).T) + bg.astype(np.float64)
    # top-k by logits == top-k by softmax probs (softmax is monotonic)
    topi = np.argpartition(-logits, TOP_K - 1, axis=1)[:, :TOP_K]  # [T, K]
    topl = np.take_along_axis(logits, topi, axis=1)
    # renormalized combine weight = masked softmax over the top-k logits
    m = topl.max(axis=1, keepdims=True)
    ex = np.exp(topl - m)
    topv = ex / ex.sum(axis=1, keepdims=True)  # [T, K]
    T = x.shape[0]
    combine = np.zeros((T, NUM_EXPERTS), np.float64)
    np.put_along_axis(combine, topi, topv, axis=1)
    idx_per_expert = [np.nonzero(combine[:, e])[0] for e in range(NUM_EXPERTS)]
    return idx_per_expert, combine.astype(np.float32)


def kernel(hidden_states, wg, bg, w1, b1, w2, b2):
    global last_exec_time_ns
    from concourse.bass_utils import run_bass_kernel_spmd

    x = np.ascontiguousarray(hidden_states, np.float32)
    wg = np.asarray(wg, np.float32)
    bg = np.asarray(bg, np.float32)
    w1 = np.asarray(w1, np.float32)
    b1 = np.asarray(b1, np.float32)
    w2 = np.asarray(w2, np.float32)
    b2 = np.asarray(b2, np.float32)
    T = x.shape[0]

    idx_per_expert, combine = _route(x, wg, bg)
    max_n = max(len(ix) for ix in idx_per_expert)
    C = max(16, -(-max_n // 16) * 16)
    assert C <= 512, f"expert capacity {C} exceeds single-matmul free dim"
    nc = _get_nc(C)

    w1p_all, w2t_all, b1p_all, b2p_all = _pack_weights(w1, b1, w2, b2)
    x_bf = x.astype(BF16)

    in_maps = []
    for c in range(N_CORES):
        # xg[e, p, k*C + c] = x[ix[c], k*128 + p]
        xg = np.zeros((EPC, P, KT, C), BF16)
        ce_arr = np.zeros((EPC, C), np.float32)
        for je in range(EPC):
            e = EPC * c + je
            ix = idx_per_expert[e]
            n = len(ix)
            if n:
                xg[je, :, :, :n] = x_bf[ix].T.reshape(KT, P, n).transpose(1, 0, 2)
                ce_arr[je, :n] = combine[ix, e]
        sl = slice(EPC * c, EPC * (c + 1))
        in_maps.append(
            {
                "xg": xg.reshape(EPC, P, KT * C),
                "w1p": w1p_all[sl],
                "w2t": w2t_all[sl],
                "b1p": b1p_all[sl],
                "b2p": b2p_all[sl],
                "ce": ce_arr,
            }
        )

    trace = bool(int(os.environ.get("KERNEL_TRACE", "0")))
    cores = list(range(N_CORES))
    try:
        r = run_bass_kernel_spmd(nc, in_maps, core_ids=cores, trace=trace)
    except Exception:
        # transient device/profiling hiccup: one clean retry without tracing
        r = run_bass_kernel_spmd(nc, in_maps, core_ids=cores, trace=False)
    last_exec_time_ns = r.exec_time_ns

    out = np.zeros((T, H), np.float32)
    for c in range(N_CORES):
        yt = np.asarray(r.results[c]["yT"], dtype=BF16)
        for je in range(EPC):
            e = EPC * c + je
            ix = idx_per_expert[e]
            n = len(ix)
            if n:
                # yt[e, p, k*C + c] = y.T[k*128 + p, c]
                ye = (
                    yt[je]
                    .reshape(P, KT, C)
                    .transpose(1, 0, 2)
                    .reshape(H, C)[:, :n]
                    .astype(np.float32)
                )
                out[ix] += ye.T
    return out


# revision 14
# speedup vs baseline: 1.2088x; 1.2088x over previous
"""GPT-OSS MoE layer (E=32 experts, top-4, H=I=1024, T=1024 tokens) on 8 TRN2
NeuronCores.

Expert-parallel sharding (4 experts/core). The host computes the router
dispatch (token->expert assignment) and performs the all-to-all gather/
scatter as part of sharding; every MLP FLOP (gate/up proj, SwiGLU, down
proj, bias adds, combine-weight scaling) runs on device.

This problem is memory-regime: the binding resource is HBM streaming of the
expert weights (48 MB/core in fp32). Weights and activations are therefore
downcast to bf16 on the host (relative error ~0.3% against a 2e-2 budget),
halving HBM traffic to ~27 MB/core. bf16 matmuls run 1 PE cycle/row at any
moving-dim size (no >=256 padding requirement like fp32r), so the moving
dim is the actual routed token capacity C. Weights stream from HBM as
[128, 2048]/[128, 1024] bf16 chunks (4 KB / 2 KB per partition line)
alternating across the two HWDGE queues (sync + scalar engines); token
activations ride SWDGE (gpsimd) as one [128, 8*C] transfer per expert in
and out, keeping descriptor lines >= 2.5 KB.
"""

import os
import sys
import types

import numpy as np
import ml_dtypes

BF16 = ml_dtypes.bfloat16

NUM_EXPERTS = 32
TOP_K = 4
H = 1024
INTER = 1024
N_CORES = 8
EPC = NUM_EXPERTS // N_CORES  # experts per core
P = 128
KT = H // P  # k tiles per contraction (8)


def _install_ntff_hook():
    """Best-effort: restore the NTFF profile hook missing from this image so
    trace=True (or BASS_TRACE=1) in run_bass_kernel_spmd can measure HW time."""
    try:
        from antenv.axon_hooks import get_axon_ntff_profile_hook  # noqa: F401

        return
    except ImportError:
        pass
    try:
        from trn_agent_boot.trn_boot import _ntff_profile_via_ctypes

        hook = _ntff_profile_via_ctypes("/opt/axon/libaxon_pjrt.so")
        mod = types.ModuleType("antenv.axon_hooks")
        mod.get_axon_ntff_profile_hook = lambda: hook
        mod.set_axon_ntff_profile_hook = lambda h: None
        sys.modules["antenv.axon_hooks"] = mod
    except Exception:
        pass


_install_ntff_hook()

_NC_CACHE = {}
last_exec_time_ns = None


def _build_nc(C):
    """Build + compile the per-core Bass program.

    C = token capacity per expert (actual routed max, rounded up to 16).
    All matmuls use moving dim C (bf16 runs full rate at any size).
    """
    import concourse.mybir as mybir
    import concourse.tile as tile
    from concourse import bacc

    dt = mybir.dt.float32
    bt = mybir.dt.bfloat16
    AF = mybir.ActivationFunctionType

    nc = bacc.Bacc(trn_type="TRN2")
    xg = nc.dram_tensor("xg", [EPC, P, KT * C], bt, kind="ExternalInput")
    # super-block-major weight layout: each 1024-column super-block's 8
    # k-chunks are contiguous in the stream, so compute consumes each 256 KB
    # chunk immediately on arrival (just-in-time) and only ~2us of work
    # remains after the last weight byte lands
    w1p = nc.dram_tensor("w1p", [EPC, 2, H, 1024], bt, kind="ExternalInput")
    w2t = nc.dram_tensor("w2t", [EPC, INTER, H], bt, kind="ExternalInput")
    b1p = nc.dram_tensor("b1p", [EPC, P, 16], dt, kind="ExternalInput")
    b2p = nc.dram_tensor("b2p", [EPC, P, 8], dt, kind="ExternalInput")
    ce = nc.dram_tensor("ce", [EPC, C], dt, kind="ExternalInput")
    yT = nc.dram_tensor("yT", [EPC, P, KT * C], bt, kind="ExternalOutput")

    with tile.TileContext(nc) as tc:
        with (
            tc.tile_pool(name="xp", bufs=EPC) as x_pool,
            tc.tile_pool(name="w1", bufs=16) as w1_pool,
            tc.tile_pool(name="w2", bufs=16) as w2_pool,
            tc.tile_pool(name="hp", bufs=16) as h_pool,
            tc.tile_pool(name="ev", bufs=4) as ev_pool,
            tc.tile_pool(name="sm", bufs=EPC) as small_pool,
            tc.tile_pool(name="yo", bufs=2) as y_pool,
            tc.tile_pool(name="ps", bufs=1, space="PSUM") as psum_pool,
        ):
            # hoist ALL token loads + per-expert smalls to the front of the
            # gpsimd queue: nothing later on SWDGE may gate compute (y stores
            # queued behind smalls would serialize the expert pipeline)
            xts, b1ts, b2ts, ce_bs = [], [], [], []
            for e in range(EPC):
                xt = x_pool.tile([P, KT * C], bt, tag="xt", name=f"xt{e}")
                nc.gpsimd.dma_start(xt[:], xg[e])
                xts.append(xt)
            for e in range(EPC):
                b1t = small_pool.tile([P, 16], dt, tag="b1t", name=f"b1t{e}")
                nc.gpsimd.dma_start(b1t[:], b1p[e])
                b1ts.append(b1t)
                b2t = small_pool.tile([P, 8], dt, tag="b2t", name=f"b2t{e}")
                nc.gpsimd.dma_start(b2t[:], b2p[e])
                b2ts.append(b2t)
            ce_rows = []
            for e in range(EPC):
                ce_row = small_pool.tile([1, C], dt, tag="ce_row", name=f"ce_row{e}")
                nc.gpsimd.dma_start(ce_row[:], ce[e : e + 1, :])
                ce_rows.append(ce_row)
            for e in range(EPC):
                # broadcast ce across partitions on gpsimd (keeps PE/PSUM free)
                ce_b = small_pool.tile([P, C], dt, tag="ce_b", name=f"ce_b{e}")
                nc.gpsimd.partition_broadcast(ce_b[:], ce_rows[e][:])
                ce_bs.append(ce_b)

            for e in range(EPC):
                xt = xts[e]
                b1t, b2t, ce_b = b1ts[e], b2ts[e], ce_bs[e]

                # ---- gate/up projection + SwiGLU (tokens in free dim) ----
                # w1p columns are packed in pair-blocks [g0 u0 g1 u1 ...];
                # each PSUM bank holds a g and u accumulator pair as slices
                h = []
                for mgp in range(2):
                    gus = [
                        psum_pool.tile([P, 2 * C], dt, tag=f"gu{i}", name=f"gu{i}")
                        for i in range(4)
                    ]
                    for k in range(KT):
                        wc = w1_pool.tile([P, 1024], bt, tag="w1c", name="w1c")
                        eng = nc.sync if (k % 2 == 0) else nc.scalar
                        eng.dma_start(wc[:], w1p[e, mgp, k * P : (k + 1) * P, :])
                        for j in range(8):
                            # j even -> g slice, j odd -> u slice of pair j//2.
                            # start=True clears the WHOLE bank, so only the
                            # first (even) slice may use it; the odd slice's
                            # k==0 matmul overwrites (has_written clear) the
                            # just-zeroed region instead
                            acc = gus[j // 2][:, (j % 2) * C : (j % 2) * C + C]
                            nc.tensor.matmul(
                                acc,
                                wc[:, j * P : (j + 1) * P],
                                xt[:, k * C : (k + 1) * C],
                                start=(k == 0 and j % 2 == 0),
                                stop=(k == KT - 1),
                                skip_group_check=(j % 2 == 1),
                            )
                    for pair in range(4):
                        jg = 8 * mgp + 2 * pair  # packed block idx of g half
                        sg = ev_pool.tile([P, C], bt, tag="sg")
                        nc.scalar.activation(
                            sg[:],
                            gus[pair][:, :C],
                            AF.Silu,
                            bias=b1t[:, jg : jg + 1],
                        )
                        us = ev_pool.tile([P, C], bt, tag="us")
                        nc.vector.tensor_scalar_add(
                            us[:], gus[pair][:, C : 2 * C], b1t[:, jg + 1 : jg + 2]
                        )
                        hm = h_pool.tile([P, C], bt, tag="h")
                        nc.vector.tensor_mul(hm[:], sg[:], us[:])
                        h.append(hm)

                # ---- down projection + bias + combine scale ----
                # all 8 H-block accumulators live at once (2 per PSUM bank):
                # every w2 chunk is consumed the moment it lands
                ystage = y_pool.tile([P, KT * C], bt, tag="ystage")
                half = KT * C // 2
                yps = [
                    psum_pool.tile([P, 2 * C], dt, tag=f"y{i}", name=f"y{i}")
                    for i in range(4)
                ]
                for k in range(KT):
                    wc = w2_pool.tile([P, 1024], bt, tag="w2c", name="w2c")
                    eng = nc.scalar if (k % 2 == 0) else nc.sync
                    eng.dma_start(wc[:], w2t[e, k * P : (k + 1) * P, :])
                    for j in range(8):
                        acc = yps[j // 2][:, (j % 2) * C : (j % 2) * C + C]
                        nc.tensor.matmul(
                            acc,
                            wc[:, j * P : (j + 1) * P],
                            h[k][:],
                            start=(k == 0 and j % 2 == 0),
                            stop=(k == KT - 1),
                            skip_group_check=(j % 2 == 1),
                        )
                for m2g in range(2):
                    for j in range(4):
                        m2 = 4 * m2g + j
                        # yo = (y + b2_col) * ce  in one DVE op
                        nc.vector.scalar_tensor_tensor(
                            ystage[:, m2 * C : (m2 + 1) * C],
                            yps[m2 // 2][:, (m2 % 2) * C : (m2 % 2) * C + C],
                            b2t[:, m2 : m2 + 1],
                            ce_b[:],
                            mybir.AluOpType.add,
                            mybir.AluOpType.mult,
                        )
                    # store each half as soon as its epilogue is done; the
                    # last expert rides the (by then idle) HWDGE queues
                    sl_ = slice(m2g * half, (m2g + 1) * half)
                    if e < EPC - 1:
                        nc.gpsimd.dma_start(yT[e, :, sl_], ystage[:, sl_])
                    else:
                        eng = nc.sync if m2g == 0 else nc.scalar
                        eng.dma_start(yT[e, :, sl_], ystage[:, sl_])

    nc.compile()
    return nc


def _get_nc(C):
    if C not in _NC_CACHE:
        _NC_CACHE[C] = _build_nc(C)
    return _NC_CACHE[C]


_PACK_CACHE = {}


def _w1_col_order():
    # packed column order for w1.T: pair blocks [g_m | u_m] of 128 channels
    return np.concatenate(
        [
            np.r_[m * P : (m + 1) * P, INTER + m * P : INTER + (m + 1) * P]
            for m in range(INTER // P)
        ]
    )


def _pack_weights(w1, b1, w2, b2):
    """Pre-transpose/pack expert weights for the device layout (bf16). Cached
    across calls on a value fingerprint so repeat invocations skip the copy."""
    key = (
        w1.shape,
        w2.shape,
        w1.reshape(-1)[:: 65537][:64].tobytes(),
        w2.reshape(-1)[:: 65537][:64].tobytes(),
        b1.reshape(-1)[:16].tobytes(),
        b2.reshape(-1)[:16].tobytes(),
    )
    if key in _PACK_CACHE:
        return _PACK_CACHE[key]
    col_order = _w1_col_order()
    # super-block-major: [E, 2, H, 1024] for w1; w2 stays k-row major
    w1p_all = np.ascontiguousarray(
        w1.transpose(0, 2, 1)[:, :, col_order]
        .reshape(NUM_EXPERTS, H, 2, 1024)
        .transpose(0, 2, 1, 3)
    ).astype(BF16)
    w2t_all = np.ascontiguousarray(w2.transpose(0, 2, 1)).astype(BF16)
    b1p_all = np.ascontiguousarray(
        b1[:, col_order].reshape(NUM_EXPERTS, 16, P).transpose(0, 2, 1)
    )
    b2p_all = np.ascontiguousarray(b2.reshape(NUM_EXPERTS, 8, P).transpose(0, 2, 1))
    _PACK_CACHE[key] = (w1p_all, w2t_all, b1p_all, b2p_all)
    return _PACK_CACHE[key]


def _route(x, wg, bg):
    """Host-side router dispatch: which experts get which tokens, and the
    renormalized combine weights (matches softmax -> top-k -> renorm)."""
    logits = (x.astype(np.float64) @ wg.astype(np.float64).T) + bg.astype(np.float64)
    # top-k by logits == top-k by softmax probs (softmax is monotonic)
    topi = np.argpartition(-logits, TOP_K - 1, axis=1)[:, :TOP_K]  # [T, K]
    topl = np.take_along_axis(logits, topi, axis=1)
    # renormalized combine weight = masked softmax over the top-k logits
    m = topl.max(axis=1, keepdims=True)
    ex = np.exp(topl - m)
    topv = ex / ex.sum(axis=1, keepdims=True)  # [T, K]
    T = x.shape[0]
    combine = np.zeros((T, NUM_EXPERTS), np.float64)
    np.put_along_axis(combine, topi, topv, axis=1)
    idx_per_expert = [np.nonzero(combine[:, e])[0] for e in range(NUM_EXPERTS)]
    return idx_per_expert, combine.astype(np.float32)


def kernel(hidden_states, wg, bg, w1, b1, w2, b2):
    global last_exec_time_ns
    from concourse.bass_utils import run_bass_kernel_spmd

    x = np.ascontiguousarray(hidden_states, np.float32)
    wg = np.asarray(wg, np.float32)
    bg = np.asarray(bg, np.float32)
    w1 = np.asarray(w1, np.float32)
    b1 = np.asarray(b1, np.float32)
    w2 = np.asarray(w2, np.float32)
    b2 = np.asarray(b2, np.float32)
    T = x.shape[0]

    idx_per_expert, combine = _route(x, wg, bg)
    max_n = max(len(ix) for ix in idx_per_expert)
    C = max(16, -(-max_n // 16) * 16)
    # two accumulators share one 2KB PSUM bank -> 2*C*4B <= 2048
    assert C <= 256, f"expert capacity {C} exceeds paired-PSUM-bank layout"
    nc = _get_nc(C)

    w1p_all, w2t_all, b1p_all, b2p_all = _pack_weights(w1, b1, w2, b2)
    x_bf = x.astype(BF16)

    in_maps = []
    for c in range(N_CORES):
        # xg[e, p, k*C + c] = x[ix[c], k*128 + p]
        xg = np.zeros((EPC, P, KT, C), BF16)
        ce_arr = np.zeros((EPC, C), np.float32)
        for je in range(EPC):
            e = EPC * c + je
            ix = idx_per_expert[e]
            n = len(ix)
            if n:
                xg[je, :, :, :n] = x_bf[ix].T.reshape(KT, P, n).transpose(1, 0, 2)
                ce_arr[je, :n] = combine[ix, e]
        sl = slice(EPC * c, EPC * (c + 1))
        in_maps.append(
            {
                "xg": xg.reshape(EPC, P, KT * C),
                "w1p": w1p_all[sl],
                "w2t": w2t_all[sl],
                "b1p": b1p_all[sl],
                "b2p": b2p_all[sl],
                "ce": ce_arr,
            }
        )

    trace = bool(int(os.environ.get("KERNEL_TRACE", "0")))
    cores = list(range(N_CORES))
    try:
        r = run_bass_kernel_spmd(nc, in_maps, core_ids=cores, trace=trace)
    except Exception:
        # transient device/profiling hiccup: one clean retry without tracing
        r = run_bass_kernel_spmd(nc, in_maps, core_ids=cores, trace=False)
    last_exec_time_ns = r.exec_time_ns

    out = np.zeros((T, H), np.float32)
    for c in range(N_CORES):
        yt = np.asarray(r.results[c]["yT"], dtype=BF16)
        for je in range(EPC):
            e = EPC * c + je
            ix = idx_per_expert[e]
            n = len(ix)
            if n:
                # yt[e, p, k*C + c] = y.T[k*128 + p, c]
                ye = (
                    yt[je]
                    .reshape(P, KT, C)
                    .transpose(1, 0, 2)
                    .reshape(H, C)[:, :n]
                    .astype(np.float32)
                )
                out[ix] += ye.T
    return out


# revision 18
# speedup vs baseline: 1.2202x; 1.0095x over previous
"""GPT-OSS MoE layer (E=32 experts, top-4, H=I=1024, T=1024 tokens) on 8 TRN2
NeuronCores.

Expert-parallel sharding (4 experts/core). The host computes the router
dispatch (token->expert assignment) and performs the all-to-all gather/
scatter as part of sharding; every MLP FLOP (gate/up proj, SwiGLU, down
proj, bias adds, combine-weight scaling) runs on device.

This problem is memory-regime: the binding resource is HBM streaming of the
expert weights (48 MB/core in fp32). Weights and activations are therefore
downcast to bf16 on the host (relative error ~0.3% against a 2e-2 budget),
halving HBM traffic to ~27 MB/core. bf16 matmuls run 1 PE cycle/row at any
moving-dim size (no >=256 padding requirement like fp32r), so the moving
dim is the actual routed token capacity C. Weights stream from HBM as
[128, 2048]/[128, 1024] bf16 chunks (4 KB / 2 KB per partition line)
alternating across the two HWDGE queues (sync + scalar engines); token
activations ride SWDGE (gpsimd) as one [128, 8*C] transfer per expert in
and out, keeping descriptor lines >= 2.5 KB.
"""

import os
import sys
import types

import numpy as np
import ml_dtypes

BF16 = ml_dtypes.bfloat16

NUM_EXPERTS = 32
TOP_K = 4
H = 1024
INTER = 1024
N_CORES = 8
EPC = NUM_EXPERTS // N_CORES  # experts per core
P = 128
KT = H // P  # k tiles per contraction (8)


def _install_ntff_hook():
    """Best-effort: restore the NTFF profile hook missing from this image so
    trace=True (or BASS_TRACE=1) in run_bass_kernel_spmd can measure HW time."""
    try:
        from antenv.axon_hooks import get_axon_ntff_profile_hook  # noqa: F401

        return
    except ImportError:
        pass
    try:
        from trn_agent_boot.trn_boot import _ntff_profile_via_ctypes

        hook = _ntff_profile_via_ctypes("/opt/axon/libaxon_pjrt.so")
        mod = types.ModuleType("antenv.axon_hooks")
        mod.get_axon_ntff_profile_hook = lambda: hook
        mod.set_axon_ntff_profile_hook = lambda h: None
        sys.modules["antenv.axon_hooks"] = mod
    except Exception:
        pass


_install_ntff_hook()

_NC_CACHE = {}
last_exec_time_ns = None


def _build_nc(C):
    """Build + compile the per-core Bass program.

    C = token capacity per expert (actual routed max, rounded up to 16).
    All matmuls use moving dim C (bf16 runs full rate at any size).
    """
    import concourse.mybir as mybir
    import concourse.tile as tile
    from concourse import bacc

    dt = mybir.dt.float32
    bt = mybir.dt.bfloat16
    AF = mybir.ActivationFunctionType

    nc = bacc.Bacc(trn_type="TRN2")
    xg = nc.dram_tensor("xg", [EPC, P, KT * C], bt, kind="ExternalInput")
    # k-major weight layout, big chunks: w1 streams as 8 x [128, 2048] bf16
    # (512 KB, 4 KB/partition line), w2 as 8 x [128, 1024] (256 KB); chunks
    # stay resident for the expert so later phases replay from SBUF at full
    # PE rate while the next expert's stream continues
    w1p = nc.dram_tensor("w1p", [EPC, H, 2 * INTER], bt, kind="ExternalInput")
    w2t = nc.dram_tensor("w2t", [EPC, INTER, H], bt, kind="ExternalInput")
    b1p = nc.dram_tensor("b1p", [EPC, P, 16], dt, kind="ExternalInput")
    b2p = nc.dram_tensor("b2p", [EPC, P, 8], dt, kind="ExternalInput")
    ce = nc.dram_tensor("ce", [EPC, C], dt, kind="ExternalInput")
    yT = nc.dram_tensor("yT", [EPC, P, KT * C], bt, kind="ExternalOutput")

    with tile.TileContext(nc) as tc:
        with (
            tc.tile_pool(name="xp", bufs=EPC) as x_pool,
            tc.tile_pool(name="w1", bufs=16) as w1_pool,
            tc.tile_pool(name="w2", bufs=16) as w2_pool,
            tc.tile_pool(name="hp", bufs=16) as h_pool,
            tc.tile_pool(name="ev", bufs=4) as ev_pool,
            tc.tile_pool(name="sm", bufs=EPC) as small_pool,
            tc.tile_pool(name="yo", bufs=2) as y_pool,
            tc.tile_pool(name="ps", bufs=1, space="PSUM") as psum_pool,
        ):
            # hoist ALL token loads + per-expert smalls to the front of the
            # gpsimd queue: nothing later on SWDGE may gate compute (y stores
            # queued behind smalls would serialize the expert pipeline)
            xts, b1ts, b2ts, ce_bs = [], [], [], []
            for e in range(EPC):
                xt = x_pool.tile([P, KT * C], bt, tag="xt", name=f"xt{e}")
                nc.gpsimd.dma_start(xt[:], xg[e])
                xts.append(xt)
            for e in range(EPC):
                b1t = small_pool.tile([P, 16], dt, tag="b1t", name=f"b1t{e}")
                nc.gpsimd.dma_start(b1t[:], b1p[e])
                b1ts.append(b1t)
                b2t = small_pool.tile([P, 8], dt, tag="b2t", name=f"b2t{e}")
                nc.gpsimd.dma_start(b2t[:], b2p[e])
                b2ts.append(b2t)
            ce_rows = []
            for e in range(EPC):
                ce_row = small_pool.tile([1, C], dt, tag="ce_row", name=f"ce_row{e}")
                nc.gpsimd.dma_start(ce_row[:], ce[e : e + 1, :])
                ce_rows.append(ce_row)
            for e in range(EPC):
                # broadcast ce across partitions on gpsimd (keeps PE/PSUM free)
                ce_b = small_pool.tile([P, C], dt, tag="ce_b", name=f"ce_b{e}")
                nc.gpsimd.partition_broadcast(ce_b[:], ce_rows[e][:])
                ce_bs.append(ce_b)

            for e in range(EPC):
                xt = xts[e]
                b1t, b2t, ce_b = b1ts[e], b2ts[e], ce_bs[e]

                # issue the whole expert's weight stream up front on the two
                # HWDGE rings; silus later in the scalar queue then never gate
                # this expert's weight arrival
                w1k = []
                for k in range(KT):
                    wc = w1_pool.tile([P, 2 * INTER], bt, tag="w1c", name="w1c")
                    eng = nc.sync if (k % 2 == 0) else nc.scalar
                    eng.dma_start(wc[:], w1p[e, k * P : (k + 1) * P, :])
                    w1k.append(wc)
                w2k = []
                for k in range(KT):
                    wc = w2_pool.tile([P, INTER], bt, tag="w2c", name="w2c")
                    eng = nc.scalar if (k % 2 == 0) else nc.sync
                    eng.dma_start(wc[:], w2t[e, k * P : (k + 1) * P, :])
                    w2k.append(wc)

                # ---- gate/up projection + SwiGLU (tokens in free dim) ----
                # w1p columns are packed in pair-blocks [g0 u0 g1 u1 ...];
                # each PSUM bank holds a g and u accumulator pair as slices
                h = []
                for mgp in range(2):
                    gus = [
                        psum_pool.tile([P, 2 * C], dt, tag=f"gu{i}", name=f"gu{i}")
                        for i in range(4)
                    ]
                    for k in range(KT):
                        wc = w1k[k][:, mgp * 1024 : (mgp + 1) * 1024]
                        for j in range(8):
                            # j even -> g slice, j odd -> u slice of pair j//2.
                            # start=True clears the WHOLE bank, so only the
                            # first (even) slice may use it; the odd slice's
                            # k==0 matmul overwrites (has_written clear) the
                            # just-zeroed region instead
                            acc = gus[j // 2][:, (j % 2) * C : (j % 2) * C + C]
                            nc.tensor.matmul(
                                acc,
                                wc[:, j * P : (j + 1) * P],
                                xt[:, k * C : (k + 1) * C],
                                start=(k == 0 and j % 2 == 0),
                                stop=(k == KT - 1),
                                skip_group_check=(j % 2 == 1),
                            )
                    for pair in range(4):
                        jg = 8 * mgp + 2 * pair  # packed block idx of g half
                        sg = ev_pool.tile([P, C], bt, tag="sg")
                        nc.scalar.activation(
                            sg[:],
                            gus[pair][:, :C],
                            AF.Silu,
                            bias=b1t[:, jg : jg + 1],
                        )
                        us = ev_pool.tile([P, C], bt, tag="us")
                        nc.vector.tensor_scalar_add(
                            us[:], gus[pair][:, C : 2 * C], b1t[:, jg + 1 : jg + 2]
                        )
                        hm = h_pool.tile([P, C], bt, tag="h")
                        nc.vector.tensor_mul(hm[:], sg[:], us[:])
                        h.append(hm)

                # ---- down projection + bias + combine scale ----
                # all 8 H-block accumulators live at once (2 per PSUM bank):
                # every w2 chunk is consumed the moment it lands
                ystage = y_pool.tile([P, KT * C], bt, tag="ystage")
                half = KT * C // 2
                yps = [
                    psum_pool.tile([P, 2 * C], dt, tag=f"y{i}", name=f"y{i}")
                    for i in range(4)
                ]
                for k in range(KT):
                    for j in range(8):
                        acc = yps[j // 2][:, (j % 2) * C : (j % 2) * C + C]
                        nc.tensor.matmul(
                            acc,
                            w2k[k][:, j * P : (j + 1) * P],
                            h[k][:],
                            start=(k == 0 and j % 2 == 0),
                            stop=(k == KT - 1),
                            skip_group_check=(j % 2 == 1),
                        )
                for m2g in range(2):
                    for j in range(4):
                        m2 = 4 * m2g + j
                        # yo = (y + b2_col) * ce  in one DVE op
                        nc.vector.scalar_tensor_tensor(
                            ystage[:, m2 * C : (m2 + 1) * C],
                            yps[m2 // 2][:, (m2 % 2) * C : (m2 % 2) * C + C],
                            b2t[:, m2 : m2 + 1],
                            ce_b[:],
                            mybir.AluOpType.add,
                            mybir.AluOpType.mult,
                        )
                    # store each half as soon as its epilogue is done; the
                    # last expert rides the (by then idle) HWDGE queues
                    sl_ = slice(m2g * half, (m2g + 1) * half)
                    if e < EPC - 1:
                        nc.gpsimd.dma_start(yT[e, :, sl_], ystage[:, sl_])
                    else:
                        eng = nc.sync if m2g == 0 else nc.scalar
                        eng.dma_start(yT[e, :, sl_], ystage[:, sl_])

    nc.compile()
    return nc


def _get_nc(C):
    if C not in _NC_CACHE:
        _NC_CACHE[C] = _build_nc(C)
    return _NC_CACHE[C]


_PACK_CACHE = {}


def _w1_col_order():
    # packed column order for w1.T: pair blocks [g_m | u_m] of 128 channels
    return np.concatenate(
        [
            np.r_[m * P : (m + 1) * P, INTER + m * P : INTER + (m + 1) * P]
            for m in range(INTER // P)
        ]
    )


def _pack_weights(w1, b1, w2, b2):
    """Pre-transpose/pack expert weights for the device layout (bf16). Cached
    across calls on a value fingerprint so repeat invocations skip the copy."""
    key = (
        w1.shape,
        w2.shape,
        w1.reshape(-1)[:: 65537][:64].tobytes(),
        w2.reshape(-1)[:: 65537][:64].tobytes(),
        b1.reshape(-1)[:16].tobytes(),
        b2.reshape(-1)[:16].tobytes(),
    )
    if key in _PACK_CACHE:
        return _PACK_CACHE[key]
    col_order = _w1_col_order()
    w1p_all = np.ascontiguousarray(w1.transpose(0, 2, 1)[:, :, col_order]).astype(BF16)
    w2t_all = np.ascontiguousarray(w2.transpose(0, 2, 1)).astype(BF16)
    b1p_all = np.ascontiguousarray(
        b1[:, col_order].reshape(NUM_EXPERTS, 16, P).transpose(0, 2, 1)
    )
    b2p_all = np.ascontiguousarray(b2.reshape(NUM_EXPERTS, 8, P).transpose(0, 2, 1))
    _PACK_CACHE[key] = (w1p_all, w2t_all, b1p_all, b2p_all)
    return _PACK_CACHE[key]


def _route(x, wg, bg):
    """Host-side router dispatch: which experts get which tokens, and the
    renormalized combine weights (matches softmax -> top-k -> renorm)."""
    logits = (x.astype(np.float64) @ wg.astype(np.float64).T) + bg.astype(np.float64)
    # top-k by logits == top-k by softmax probs (softmax is monotonic)
    topi = np.argpartition(-logits, TOP_K - 1, axis=1)[:, :TOP_K]  # [T, K]
    topl = np.take_along_axis(logits, topi, axis=1)
    # renormalized combine weight = masked softmax over the top-k logits
    m = topl.max(axis=1, keepdims=True)
    ex = np.exp(topl - m)
    topv = ex / ex.sum(axis=1, keepdims=True)  # [T, K]
    T = x.shape[0]
    combine = np.zeros((T, NUM_EXPERTS), np.float64)
    np.put_along_axis(combine, topi, topv, axis=1)
    idx_per_expert = [np.nonzero(combine[:, e])[0] for e in range(NUM_EXPERTS)]
    return idx_per_expert, combine.astype(np.float32)


def kernel(hidden_states, wg, bg, w1, b1, w2, b2):
    global last_exec_time_ns
    from concourse.bass_utils import run_bass_kernel_spmd

    x = np.ascontiguousarray(hidden_states, np.float32)
    wg = np.asarray(wg, np.float32)
    bg = np.asarray(bg, np.float32)
    w1 = np.asarray(w1, np.float32)
    b1 = np.asarray(b1, np.float32)
    w2 = np.asarray(w2, np.float32)
    b2 = np.asarray(b2, np.float32)
    T = x.shape[0]

    idx_per_expert, combine = _route(x, wg, bg)
    max_n = max(len(ix) for ix in idx_per_expert)
    C = max(16, -(-max_n // 16) * 16)
    # two accumulators share one 2KB PSUM bank -> 2*C*4B <= 2048
    assert C <= 256, f"expert capacity {C} exceeds paired-PSUM-bank layout"
    nc = _get_nc(C)

    w1p_all, w2t_all, b1p_all, b2p_all = _pack_weights(w1, b1, w2, b2)
    x_bf = x.astype(BF16)

    in_maps = []
    for c in range(N_CORES):
        # xg[e, p, k*C + c] = x[ix[c], k*128 + p]
        xg = np.zeros((EPC, P, KT, C), BF16)
        ce_arr = np.zeros((EPC, C), np.float32)
        for je in range(EPC):
            e = EPC * c + je
            ix = idx_per_expert[e]
            n = len(ix)
            if n:
                xg[je, :, :, :n] = x_bf[ix].T.reshape(KT, P, n).transpose(1, 0, 2)
                ce_arr[je, :n] = combine[ix, e]
        sl = slice(EPC * c, EPC * (c + 1))
        in_maps.append(
            {
                "xg": xg.reshape(EPC, P, KT * C),
                "w1p": w1p_all[sl],
                "w2t": w2t_all[sl],
                "b1p": b1p_all[sl],
                "b2p": b2p_all[sl],
                "ce": ce_arr,
            }
        )

    trace = bool(int(os.environ.get("KERNEL_TRACE", "0")))
    cores = list(range(N_CORES))
    try:
        r = run_bass_kernel_spmd(nc, in_maps, core_ids=cores, trace=trace)
    except Exception:
        # transient device/profiling hiccup: one clean retry without tracing
        r = run_bass_kernel_spmd(nc, in_maps, core_ids=cores, trace=False)
    last_exec_time_ns = r.exec_time_ns

    out = np.zeros((T, H), np.float32)
    for c in range(N_CORES):
        yt = np.asarray(r.results[c]["yT"], dtype=BF16)
        for je in range(EPC):
            e = EPC * c + je
            ix = idx_per_expert[e]
            n = len(ix)
            if n:
                # yt[e, p, k*C + c] = y.T[k*128 + p, c]
                ye = (
                    yt[je]
                    .reshape(P, KT, C)
                    .transpose(1, 0, 2)
                    .reshape(H, C)[:, :n]
                    .astype(np.float32)
                )
                out[ix] += ye.T
    return out
